# revision 1
# baseline (speedup 1.0000x reference)
"""Correlation kernel for Trainium2 (Bass/Tile), 8 NeuronCores.

Problem: inputs (B=4, N=2, C=128, H=128, W=128) fp32.
  src = inputs[:, 0], target = inputs[:, 1]
  out[b, k, y, x] = (1/C) * sum_c src[b,c,y,x] * target[b,c,y+dy,x+dx]
  for k = (dy+10)*21 + (dx+10), dy,dx in [-10,10], zero-padded target.
  Output (4, 441, 128, 128) fp32.

Mapping:
  - Shard over 8 cores: (b in 0..3) x (H half in 0..1). Each core handles
    64 output rows; halos come from host-side padded slabs.
  - Per output row y, contraction over C runs on the PE:
      stationary = src row chunk (128c x 32x), col-tiled at tile_position
      (0, 32g) so 4 x-chunks share the array;
      moving = padded target rows [y+dy', x-window 52 wide], 7 dy per
      matmul (N = 7*52 = 364 <= 512, one PSUM bank).
    PSUM tile per y: [128, 3, 512] (3 banks).
    Numerics: inputs are split on the host into bf16 hi + lo halves and
    the product is computed as hi*hi + hi*lo + lo*hi (3 accumulating
    matmuls, ~1e-5 rel err); bf16 is the fast/safe PE path.
  - DVE/ScalarE evacuate PSUM -> SBUF; 4 rows are batched per output DMA
    (2.2 MB transfers) into a (64,128,1092) per-core "window" tensor.
  - Host extracts the 21 needed diagonals per 52-wide window
    (out[..., dx] = win[..., (x mod 32) + dx]) while unsharding.
Scaling by 1/C is folded into src on the host (exact: 2^-7).
"""

import os

import ml_dtypes
import numpy as np

import concourse.bacc as bacc
import concourse.bass as bass
import concourse.mybir as mybir
import concourse.tile as tile
from concourse.bass_utils import run_bass_kernel_spmd

B = 4
C = 128
H = 128
W = 128
KS = 21          # kernel size (per axis)
P = KS // 2      # pad / max displacement = 10
HY = H // 2      # rows per core = 64
NG = 4           # x groups (col-tiling), 32 wide each
GW = 32          # group width
WIN = GW + 2 * P     # 52: target x-window per group
DYB = 3          # dy batches
DYI = KS // DYB  # 7 dy per batch
NMOV = DYI * WIN     # 364 moving columns per matmul
TGT_H = HY + 2 * P   # 84 target rows per core
TGT_W = W + 2 * P    # 148 padded target width
OUTF = DYB * NMOV    # 1092 values per (y, x)
YB = 4               # output rows per store DMA

_CACHE = {}


def _build_module(mode: str):
    """Build the SPMD Bass module (same program on all 8 cores)."""
    f32 = mybir.dt.float32
    bf16 = mybir.dt.bfloat16
    nc = bacc.Bacc("TRN2", target_bir_lowering=False, debug=False)

    split = mode.startswith("bf16")
    in_dt = bf16 if split else f32
    src_names = ["src_hi", "src_lo"] if mode == "bf16x3" else ["src_hi"]
    tgt_names = ["tgt_hi", "tgt_lo"] if mode == "bf16x3" else ["tgt_hi"]

    src_d = {n: nc.declare_dram_parameter(n, [C, HY, W], in_dt, isOutput=False)
             for n in src_names}
    tgt_d = {n: nc.declare_dram_parameter(n, [C, TGT_H, TGT_W], in_dt, isOutput=False)
             for n in tgt_names}
    out_d = nc.declare_dram_parameter("out_win", [HY, 128, OUTF], f32, isOutput=True)

    mm_dt = mybir.dt.float32r if mode == "fp32r" else in_dt

    with tile.TileContext(nc) as tc:
        with (
            tc.tile_pool(name="inp", bufs=1) as inp,
            tc.tile_pool(name="psum", bufs=2, space=bass.MemorySpace.PSUM) as psum,
            tc.tile_pool(name="win", bufs=4) as winp,
        ):
            src_sb = {n: inp.tile([C, HY, W], in_dt, name=f"sb_{n}")
                      for n in src_names}
            tgt_sb = {n: inp.tile([C, TGT_H, TGT_W], in_dt, name=f"sb_{n}")
                      for n in tgt_names}
            # Split loads so early rows' matmuls can start before the whole
            # slab lands.
            nchunk = 8
            for i in range(nchunk):
                ys = (TGT_H + nchunk - 1) // nchunk
                lo = i * ys
                hi = min(TGT_H, lo + ys)
                for n in tgt_names:
                    nc.sync.dma_start(tgt_sb[n][:, lo:hi, :], tgt_d[n][:, lo:hi, :])
                ys = (HY + nchunk - 1) // nchunk
                lo = i * ys
                hi = min(HY, lo + ys)
                for n in src_names:
                    nc.sync.dma_start(src_sb[n][:, lo:hi, :], src_d[n][:, lo:hi, :])

            if mode == "bf16x3":
                passes = [("src_hi", "tgt_hi"), ("src_hi", "tgt_lo"),
                          ("src_lo", "tgt_hi")]
            else:
                passes = [("src_hi", "tgt_hi")]

            for yb in range(HY // YB):
                win = winp.tile([128, YB, DYB, NMOV], f32)
                for yy in range(YB):
                    y = yb * YB + yy
                    ps = psum.tile([128, DYB, 512], f32)
                    # pass-major inside each dy batch: consecutive matmuls
                    # hit different col-strips, so LDWEIGHTS prefetch hides
                    # behind the previous strip's matmul.
                    for dyb in range(DYB):
                        for ip, (sn, tn) in enumerate(passes):
                            for g in range(NG):
                                lhsT = src_sb[sn][:, y, g * GW:(g + 1) * GW]
                                rhs = tgt_sb[tn][:, y + dyb * DYI:
                                                 y + (dyb + 1) * DYI,
                                                 g * GW: g * GW + WIN]
                                nc.tensor.matmul(
                                    ps[g * GW:(g + 1) * GW, dyb, 0:NMOV],
                                    lhsT.bitcast(mm_dt),
                                    rhs.bitcast(mm_dt),
                                    start=(ip == 0),
                                    stop=(ip == len(passes) - 1),
                                    tile_position=(0, g * GW),
                                )
                    if y % 2 == 0:
                        nc.vector.tensor_copy(win[:, yy], ps[:, :, 0:NMOV])
                    else:
                        nc.scalar.copy(win[:, yy], ps[:, :, 0:NMOV])
                nc.sync.dma_start(
                    out_d[yb * YB:(yb + 1) * YB].rearrange("y p f -> p y f"),
                    win[:].rearrange("p y a b -> p y (a b)"),
                )

    nc.compile()
    return nc


def _get_module(mode: str):
    if mode not in _CACHE:
        _CACHE[mode] = _build_module(mode)
    return _CACHE[mode]


def _split_bf16(x):
    hi = x.astype(ml_dtypes.bfloat16)
    lo = (x - hi.astype(np.float32)).astype(ml_dtypes.bfloat16)
    return hi, lo


def _shard_inputs(inputs: np.ndarray, mode: str):
    src = np.ascontiguousarray(inputs[:, 0]) * np.float32(1.0 / C)  # exact
    tgt = inputs[:, 1]
    tgt_pad = np.pad(tgt, ((0, 0), (0, 0), (P, P), (P, P)))
    in_maps = []
    for core in range(8):
        b, h = divmod(core, 2)
        s = np.ascontiguousarray(src[b, :, h * HY:(h + 1) * HY, :])
        t = np.ascontiguousarray(tgt_pad[b, :, h * HY: h * HY + TGT_H, :])
        if mode.startswith("bf16"):
            s_hi, s_lo = _split_bf16(s)
            t_hi, t_lo = _split_bf16(t)
            m = {"src_hi": s_hi, "tgt_hi": t_hi}
            if mode == "bf16x3":
                m["src_lo"] = s_lo
                m["tgt_lo"] = t_lo
        else:
            m = {"src_hi": s, "tgt_hi": t}
        in_maps.append(m)
    return in_maps


# (x mod 32) + dx' index into the 52-wide window, for each (x, dx')
_XIDX = (np.arange(128) % GW)[:, None] + np.arange(KS)[None, :]  # (128, 21)


def _extract(win: np.ndarray) -> np.ndarray:
    """(HY, 128, OUTF) window tensor -> (441, HY, 128) output block."""
    w4 = win.reshape(HY, 128, KS, WIN)  # [y, x, dy', u]
    idx = np.broadcast_to(_XIDX[None, :, None, :], (HY, 128, KS, KS))
    o4 = np.take_along_axis(w4, idx, axis=3)  # [y, x, dy', dx']
    return o4.transpose(2, 3, 0, 1).reshape(KS * KS, HY, 128)


def run(inputs: np.ndarray, trace: bool = False, mode: str | None = None):
    if mode is None:
        mode = os.environ.get("CORR_MM_MODE", "bf16x3")
    nc = _get_module(mode)
    in_maps = _shard_inputs(inputs, mode)
    res = run_bass_kernel_spmd(
        nc, in_maps, core_ids=list(range(8)), trace=trace,
    )
    out = np.empty((B, KS * KS, H, W), dtype=np.float32)
    for core in range(8):
        b, h = divmod(core, 2)
        out[b, :, h * HY:(h + 1) * HY, :] = _extract(res.results[core]["out_win"])
    return out, res.exec_time_ns


def kernel(inputs: np.ndarray) -> np.ndarray:
    out, _ = run(np.asarray(inputs))
    return out

